# revision 6
# baseline (speedup 1.0000x reference)
"""DGC layer (graph conv with normalized Laplacian) on 8 Trainium2 NeuronCores.

Computes out = relu((I - D^-1/2 A_norm D^-1/2) @ (H @ W)) with
A_norm = relu((A + A.T)/2), sharded row-wise over 8 cores.

Math notes (per core, row block R of 512 rows):
  - A_norm is symmetric, so its column block A_norm[:, R] equals
    A_norm[R, :].T - exactly the layout the tensor engine wants for
    contracting over the full node dim.
  - The 0.5 symmetrization factor cancels out of D^-1/2 A_norm D^-1/2,
    so we use S = relu(A + A.T) and d = rowsum(S) instead.
  - out[R] = relu(HW[R] - dis[R] * (M.T @ G)) where M = S[:, R] (bf16),
    G[c] = dis[c] * HW[c] (bf16, AllGathered), HW[R] kept in fp32 so the
    dominant term has full precision.
"""

import sys

sys.path.insert(0, "/opt/trn_rl_repo")

import numpy as np

B, N, F = 8, 4096, 256
N_CORES = 8
RB = N // N_CORES          # 512 rows per core
NT = N // 128              # 32 contraction tiles of 128
RS = RB // 128             # 4 row subtiles per core
FT = F // 128              # 2 f_in tiles


def _build_kernel(repeat=1):
    import concourse.bass as bass
    import concourse.mybir as mybir
    import concourse.tile as tile
    from concourse import bacc
    from concourse.masks import make_identity

    f32 = mybir.dt.float32
    bf16 = mybir.dt.bfloat16

    nc = bacc.Bacc(num_devices=N_CORES)

    arows = nc.declare_dram_parameter("Arows", [RB, N], f32, isOutput=False)
    acols = nc.declare_dram_parameter("Acols", [N, RB], f32, isOutput=False)
    hr = nc.declare_dram_parameter("Hr", [B, RB, F], f32, isOutput=False)
    w = nc.declare_dram_parameter("W", [F, F], f32, isOutput=False)
    out_r = nc.declare_dram_parameter("OutR", [B, RB, F], f32, isOutput=True)

    with tile.TileContext(nc) as tc:
        with (
            tc.tile_pool(name="singles", bufs=1) as singles,
            tc.tile_pool(name="persist", bufs=1) as persist,
            tc.tile_pool(name="dram", bufs=1, space="DRAM") as dram,
        ):
            identity = singles.tile([128, 128], f32)
            make_identity(nc, identity[:])
            ones_bf = singles.tile([128, 1], bf16)
            nc.vector.memset(ones_bf[:], 1.0)
            ones4 = singles.tile([128, 4], f32)
            nc.vector.memset(ones4[:], 1.0)

            # W in [f_in (part), ft, f_out] layout for the H@W matmul
            w_sb = singles.tile([128, FT, F], f32)
            nc.sync.dma_start(out=w_sb[:], in_=w.rearrange("(t p) o -> p t o", p=128))

            # persistent blocks
            m_all = persist.tile([128, NT, RB], bf16)       # S[:, R], c on partitions
            hwr = persist.tile([128, B, RS, F], f32)        # HW[b, R, f] fp32
            dis4 = persist.tile([128, 4], f32)              # dis[R] as [p, rs]
            neg_dis4 = persist.tile([128, 4], f32)

            for _rep in range(repeat):
                _kernel_body(nc, tc, mybir, singles, dram, persist,
                             identity, ones_bf, ones4, w_sb,
                             m_all, hwr, dis4, neg_dis4,
                             arows, acols, hr, out_r)

    nc.compile()
    return nc


def _kernel_body(nc, tc, mybir, singles, dram, persist,
                 identity, ones_bf, ones4, w_sb,
                 m_all, hwr, dis4, neg_dis4,
                 arows, acols, hr, out_r):
    f32 = mybir.dt.float32
    bf16 = mybir.dt.bfloat16
    if True:
        if True:
            ag_in = dram.tile([B, RB, F], bf16, tag="ag_in")
            ag_out = dram.tile(
                [N_CORES, B, RB, F], bf16, addr_space="Shared", tag="ag_out"
            )

            # ---------------- Phase 1: HW = H @ W (fp32), and M build ----------
            with (
                tc.tile_pool(name="arst", bufs=1) as arst,
                tc.tile_pool(name="acolp", bufs=3) as acolp,
                tc.tile_pool(name="hp", bufs=2) as hp,
                tc.tile_pool(name="htp", bufs=2) as htp,
                tc.tile_pool(name="mtmpp", bufs=2) as mtmpp,
                tc.tile_pool(name="ps_tr", bufs=3, space="PSUM") as ps_tr,
                tc.tile_pool(name="ps_hw", bufs=2, space="PSUM") as ps_hw,
                tc.tile_pool(name="ps_d", bufs=1, space="PSUM") as ps_d,
            ):
                # --- H @ W per batch: transpose H tiles, matmul with W ---
                for b in range(B):
                    h_b = hp.tile([128, RS, F], f32, tag="h_b")
                    nc.sync.dma_start(
                        out=h_b[:], in_=hr[b].rearrange("(j p) f -> p j f", p=128)
                    )
                    ht_b = htp.tile([128, FT, RB], f32, tag="ht_b")
                    for rs in range(RS):
                        for ft in range(FT):
                            pst = ps_tr.tile([128, 128], f32, tag="pst")
                            nc.tensor.transpose(
                                pst[:],
                                h_b[:, rs, ft * 128 : (ft + 1) * 128],
                                identity[:],
                            )
                            nc.scalar.copy(
                                ht_b[:, ft, rs * 128 : (rs + 1) * 128], pst[:]
                            )
                    for rs in range(RS):
                        phw = ps_hw.tile([128, F], f32, tag="phw")
                        for ft in range(FT):
                            nc.tensor.matmul(
                                phw[:],
                                lhsT=ht_b[:, ft, rs * 128 : (rs + 1) * 128],
                                rhs=w_sb[:, ft, :],
                                start=(ft == 0),
                                stop=(ft == FT - 1),
                            )
                        nc.vector.tensor_copy(out=hwr[:, b, rs, :], in_=phw[:])

                # --- M = relu(Acols + Arows^T), bf16; d = colsum via ones matmul ---
                ars_sb = arst.tile([128, RS, N], f32)
                for rs in range(RS):
                    nc.sync.dma_start(
                        out=ars_sb[:, rs, :], in_=arows[rs * 128 : (rs + 1) * 128, :]
                    )
                d_ps = ps_d.tile([1, RB], f32)
                for ct in range(NT):
                    acol_t = acolp.tile([128, RB], f32, tag="acol")
                    nc.sync.dma_start(
                        out=acol_t[:], in_=acols[ct * 128 : (ct + 1) * 128, :]
                    )
                    mtmp = mtmpp.tile([128, RB], bf16, tag="mtmp")
                    for rs in range(RS):
                        pst = ps_tr.tile([128, 128], f32, tag="pst")
                        nc.tensor.transpose(
                            pst[:],
                            ars_sb[:, rs, ct * 128 : (ct + 1) * 128],
                            identity[:],
                        )
                        nc.vector.tensor_tensor(
                            out=mtmp[:, rs * 128 : (rs + 1) * 128],
                            in0=acol_t[:, rs * 128 : (rs + 1) * 128],
                            in1=pst[:],
                            op=mybir.AluOpType.add,
                        )
                    nc.scalar.activation(
                        out=m_all[:, ct, :],
                        in_=mtmp[:],
                        func=mybir.ActivationFunctionType.Relu,
                    )
                    nc.tensor.matmul(
                        d_ps[:],
                        lhsT=ones_bf[:],
                        rhs=m_all[:, ct, :],
                        start=(ct == 0),
                        stop=(ct == NT - 1),
                    )

                # --- dis = where(d > 0, 1/sqrt(d), 0), shaped [128, rs] ---
                d_sb = singles.tile([1, RB], f32)
                nc.scalar.copy(d_sb[:], d_ps[:])
                dps_t = ps_tr.tile([128, 4], f32, tag="pst")
                for rs in range(RS):
                    nc.tensor.transpose(
                        dps_t[:, rs : rs + 1],
                        d_sb[0:1, rs * 128 : (rs + 1) * 128],
                        identity[0:1, 0:1],
                    )
                dT = singles.tile([128, 4], f32)
                nc.vector.tensor_copy(out=dT[:], in_=dps_t[:])
                mask4 = singles.tile([128, 4], mybir.dt.uint8)
                nc.vector.tensor_scalar(
                    out=mask4[:],
                    in0=dT[:],
                    scalar1=0.0,
                    scalar2=None,
                    op0=mybir.AluOpType.is_gt,
                )
                maskf = singles.tile([128, 4], f32)
                nc.vector.tensor_scalar(
                    out=maskf[:],
                    in0=dT[:],
                    scalar1=0.0,
                    scalar2=None,
                    op0=mybir.AluOpType.is_gt,
                )
                dsafe = singles.tile([128, 4], f32)
                nc.vector.select(dsafe[:], mask4[:], dT[:], ones4[:])
                rcp4 = singles.tile([128, 4], f32)
                nc.vector.reciprocal(rcp4[:], dsafe[:])
                srt4 = singles.tile([128, 4], f32)
                nc.scalar.activation(
                    srt4[:], rcp4[:], mybir.ActivationFunctionType.Sqrt
                )
                nc.vector.tensor_tensor(
                    out=dis4[:], in0=srt4[:], in1=maskf[:], op=mybir.AluOpType.mult
                )
                nc.vector.tensor_scalar_mul(neg_dis4[:], dis4[:], -1.0)

                # --- G_R = dis[R] * HW[R] in bf16, ship to AllGather input ---
                for b in range(B):
                    gr_b = mtmpp.tile([128, RS, F], bf16, tag="gr_b")
                    for rs in range(RS):
                        nc.vector.tensor_scalar_mul(
                            gr_b[:, rs, :], hwr[:, b, rs, :], dis4[:, rs : rs + 1]
                        )
                    nc.sync.dma_start(
                        out=ag_in[b].rearrange("(j p) f -> p j f", p=128), in_=gr_b[:]
                    )

            nc.gpsimd.collective_compute(
                "AllGather",
                mybir.AluOpType.bypass,
                replica_groups=[list(range(N_CORES))],
                ins=[ag_in.opt()],
                outs=[ag_out.opt()],
            )

            # ---------------- Phase 3: out[R] = relu(HW[R] - dis*(M^T @ G)) ----
            with (
                tc.tile_pool(name="gp", bufs=2) as gp,
                tc.tile_pool(name="epi", bufs=4) as epi,
                tc.tile_pool(name="outp", bufs=4) as outp,
                tc.tile_pool(name="ps_mm", bufs=4, space="PSUM") as ps_mm,
            ):
                for pair in range(B // 2):
                    g_pair = gp.tile([128, NT, 2, F], bf16, tag="g_pair")
                    for rank in range(N_CORES):
                        for bp in range(2):
                            b = pair * 2 + bp
                            nc.sync.dma_start(
                                out=g_pair[:, rank * 4 : (rank + 1) * 4, bp, :],
                                in_=ag_out[rank, b].rearrange(
                                    "(j p) f -> p j f", p=128
                                ),
                            )
                    for rs in range(RS):
                        pmm = ps_mm.tile([128, 2, F], f32, tag="pmm")
                        for ct in range(NT):
                            nc.tensor.matmul(
                                pmm[:],
                                lhsT=m_all[:, ct, rs * 128 : (rs + 1) * 128],
                                rhs=g_pair[:, ct, :, :],
                                start=(ct == 0),
                                stop=(ct == NT - 1),
                            )
                        for bp in range(2):
                            b = pair * 2 + bp
                            t1 = epi.tile([128, F], f32, tag="t1")
                            nc.vector.scalar_tensor_tensor(
                                out=t1[:],
                                in0=pmm[:, bp, :],
                                scalar=neg_dis4[:, rs : rs + 1],
                                in1=hwr[:, b, rs, :],
                                op0=mybir.AluOpType.mult,
                                op1=mybir.AluOpType.add,
                            )
                            o_t = outp.tile([128, F], f32, tag="o_t")
                            nc.scalar.activation(
                                o_t[:], t1[:], mybir.ActivationFunctionType.Relu
                            )
                            nc.sync.dma_start(
                                out=out_r[b, rs * 128 : (rs + 1) * 128, :], in_=o_t[:]
                            )


_NC_CACHE = None


def kernel(H, W, A):
    global _NC_CACHE
    from concourse.bass_utils import run_bass_kernel_spmd

    H = np.asarray(H, dtype=np.float32)
    W = np.asarray(W, dtype=np.float32)
    A = np.asarray(A, dtype=np.float32)

    if _NC_CACHE is None:
        _NC_CACHE = _build_kernel()
    nc = _NC_CACHE

    in_maps = []
    for c in range(N_CORES):
        r0, r1 = c * RB, (c + 1) * RB
        in_maps.append(
            {
                "Arows": np.ascontiguousarray(A[r0:r1, :]),
                "Acols": np.ascontiguousarray(A[:, r0:r1]),
                "Hr": np.ascontiguousarray(H[:, r0:r1, :]),
                "W": W,
            }
        )

    res = run_bass_kernel_spmd(nc, in_maps, list(range(N_CORES)))

    out = np.empty((B, N, F), dtype=np.float32)
    for c in range(N_CORES):
        out[:, c * RB : (c + 1) * RB, :] = res.results[c]["OutR"]
    return out


if __name__ == "__main__":
    rng = np.random.default_rng(0)
    H = rng.standard_normal((B, N, F)).astype(np.float32)
    W = rng.standard_normal((F, F)).astype(np.float32) / 16.0
    A = rng.standard_normal((N, N)).astype(np.float32) * 0.0262
    out = kernel(H, W, A)
    print("kernel ran, out shape", out.shape)


# revision 12
# speedup vs baseline: 1.1127x; 1.1127x over previous
"""DGC layer (graph conv with normalized Laplacian) on 8 Trainium2 NeuronCores.

Computes out = relu((I - D^-1/2 A_norm D^-1/2) @ (H @ W)) with
A_norm = relu((A + A.T)/2), sharded row-wise over 8 cores.

Math notes (per core, row block R of 512 rows):
  - A_norm is symmetric, so its column block A_norm[:, R] equals
    A_norm[R, :].T - exactly the layout the tensor engine wants for
    contracting over the full node dim.
  - The 0.5 symmetrization factor cancels out of D^-1/2 A_norm D^-1/2,
    so we use S = relu(A + A.T) and d = rowsum(S) instead.
  - out[R] = relu(HW[R] - dis[R] * (M'.T @ HWg)) where M' = dis_c * S[:, R]
    (bf16, column scaling folded into the stationary operand), HWg = the
    AllGathered unscaled bf16 HW, and HW[R] kept in fp32 so the dominant
    term has full precision.
  - The HW AllGather only depends on H @ W, so it runs concurrently with
    the whole A_norm build. Degrees for all nodes come from AllReduces of
    per-core partial column sums (free via the relu's accum_out), split in
    two halves so the first half of the big matmul can start while the
    second half of M is still being built.
"""

import sys

sys.path.insert(0, "/opt/trn_rl_repo")

import numpy as np

B, N, F = 8, 4096, 256
N_CORES = 8
RB = N // N_CORES          # 512 rows per core
NT = N // 128              # 32 contraction tiles of 128
NH = NT // 2               # 16 tiles per degree-reduce half
RS = RB // 128             # 4 row subtiles per core
FT = F // 128              # 2 f_in tiles
_SIM_LOCAL_AG = False      # analyze.py sets True (fake collective, Local DRAM)


def _build_kernel(repeat=1):
    import concourse.mybir as mybir
    import concourse.tile as tile
    from concourse import bacc
    from concourse.masks import make_identity

    f32 = mybir.dt.float32
    bf16 = mybir.dt.bfloat16

    nc = bacc.Bacc(num_devices=N_CORES)

    arows = nc.declare_dram_parameter("Arows", [RB, N], f32, isOutput=False)
    acols = nc.declare_dram_parameter("Acols", [N, RB], f32, isOutput=False)
    hr = nc.declare_dram_parameter("Hr", [B, RB, F], f32, isOutput=False)
    w = nc.declare_dram_parameter("W", [F, F], f32, isOutput=False)
    out_r = nc.declare_dram_parameter("OutR", [B, RB, F], f32, isOutput=True)

    with tile.TileContext(nc) as tc:
        with (
            tc.tile_pool(name="singles", bufs=1) as singles,
            tc.tile_pool(name="persist", bufs=1) as persist,
            tc.tile_pool(name="dram", bufs=1, space="DRAM") as dram,
        ):
            identity = singles.tile([128, 128], f32)
            make_identity(nc, identity[:])
            ones_bf = singles.tile([128, 1], bf16)
            nc.vector.memset(ones_bf[:], 1.0)
            ones4 = singles.tile([128, 4], f32)
            nc.vector.memset(ones4[:], 1.0)
            ones32 = singles.tile([128, NT], f32)
            nc.vector.memset(ones32[:], 1.0)

            # W in [f_in (part), ft, f_out] layout, float32r for the fast matmul
            w_sb = singles.tile([128, FT, F], mybir.dt.float32r)
            nc.sync.dma_start(
                out=w_sb[:],
                in_=w.rearrange("(t p) o -> p t o", p=128).bitcast(
                    mybir.dt.float32r
                ),
            )

            # persistent blocks
            m_all = persist.tile([128, NT, RB], bf16)       # S[:, R], c on partitions
            hwr = persist.tile([128, B, RS, F], f32)        # HW[b, R, f] fp32
            pd = persist.tile([128, NT], f32)               # partial col sums
            dis4 = persist.tile([128, 4], f32)              # dis[R] as [p, rs]
            neg_dis4 = persist.tile([128, 4], f32)
            disall = persist.tile([128, NT], f32)           # dis[c] as [p, ct]

            for _rep in range(repeat):
                _kernel_body(nc, tc, mybir, singles, dram,
                             identity, ones_bf, ones4, ones32, w_sb,
                             m_all, hwr, pd, dis4, neg_dis4, disall,
                             arows, acols, hr, out_r)

    nc.compile()
    return nc


def _kernel_body(nc, tc, mybir, singles, dram,
                 identity, ones_bf, ones4, ones32, w_sb,
                 m_all, hwr, pd, dis4, neg_dis4, disall,
                 arows, acols, hr, out_r):
    f32 = mybir.dt.float32
    f32r = mybir.dt.float32r
    bf16 = mybir.dt.bfloat16

    ag_in = dram.tile([B, RB, F], bf16, tag="ag_in")
    ag_out = dram.tile(
        [N_CORES, B, RB, F],
        bf16,
        addr_space="Local" if _SIM_LOCAL_AG else "Shared",
        tag="ag_out",
    )
    pd_in = dram.tile([128, NT], f32, tag="pd_in")
    pd_out = dram.tile(
        [128, NT], f32,
        addr_space="Local" if _SIM_LOCAL_AG else "Shared", tag="pd_out",
    )

    # ---------- Phase A: HW = H @ W (float32r), cast to bf16, AllGather ----
    with (
        tc.tile_pool(name="hp", bufs=2) as hp,
        tc.tile_pool(name="htp", bufs=2) as htp,
        tc.tile_pool(name="ps_tr_h", bufs=2, space="PSUM") as ps_tr_h,
        tc.tile_pool(name="ps_hw", bufs=2, space="PSUM") as ps_hw,
    ):
        for b in range(B):
            h_b = hp.tile([128, RS, F], f32, tag="h_b")
            nc.sync.dma_start(
                out=h_b[:], in_=hr[b].rearrange("(j p) f -> p j f", p=128)
            )
            ht_b = htp.tile([128, FT, RB], f32r, tag="ht_b")
            for rs in range(RS):
                for ft in range(FT):
                    pst = ps_tr_h.tile([128, 128], f32, tag="psth")
                    nc.tensor.transpose(
                        pst[:],
                        h_b[:, rs, ft * 128 : (ft + 1) * 128],
                        identity[:],
                    )
                    nc.scalar.copy(
                        ht_b[:, ft, rs * 128 : (rs + 1) * 128], pst[:]
                    )
            for rs in range(RS):
                phw = ps_hw.tile([128, F], f32, tag="phw")
                for ft in range(FT):
                    nc.tensor.matmul(
                        phw[:],
                        lhsT=ht_b[:, ft, rs * 128 : (rs + 1) * 128],
                        rhs=w_sb[:, ft, :],
                        start=(ft == 0),
                        stop=(ft == FT - 1),
                    )
                nc.vector.tensor_copy(out=hwr[:, b, rs, :], in_=phw[:])
            # bf16 cast during DMA (SWDGE) straight into the gather input
            nc.gpsimd.dma_start(
                out=ag_in[b].rearrange("(j p) f -> p j f", p=128),
                in_=hwr[:, b, :, :],
            )

    nc.gpsimd.collective_compute(
        "AllGather",
        mybir.AluOpType.bypass,
        replica_groups=[list(range(N_CORES))],
        ins=[ag_in.opt()],
        outs=[ag_out.opt()],
    )

    # ---------- Phase B: M = relu(Acols + Arows^T) bf16, degrees ----------
    with (
        tc.tile_pool(name="arst", bufs=1) as arst,
        tc.tile_pool(name="acolp", bufs=3) as acolp,
        tc.tile_pool(name="mtmpp", bufs=3) as mtmpp,
        tc.tile_pool(name="ps_tr", bufs=4, space="PSUM") as ps_tr,
        tc.tile_pool(name="ps_d", bufs=1, space="PSUM") as ps_d,
    ):
        ars_sb = arst.tile([128, RS, N], f32)
        for rs in range(RS):
            nc.sync.dma_start(
                out=ars_sb[:, rs, :], in_=arows[rs * 128 : (rs + 1) * 128, :]
            )

        CC = 4                      # acol tiles per DMA chunk (1 MiB)
        for cc in range(NT // CC):
            acol_t = acolp.tile([128, CC, RB], f32, tag="acol")
            nc.sync.dma_start(
                out=acol_t[:],
                in_=acols[cc * CC * 128 : (cc + 1) * CC * 128, :]
                .rearrange("(c p) r -> p c r", p=128),
            )
            for ci in range(CC):
                ct = cc * CC + ci
                mtmp = mtmpp.tile([128, RB], bf16, tag="mtmp")
                for rs in range(RS):
                    pst = ps_tr.tile([128, 128], f32, tag="pst")
                    nc.tensor.transpose(
                        pst[:],
                        ars_sb[:, rs, ct * 128 : (ct + 1) * 128],
                        identity[:],
                    )
                    nc.vector.tensor_tensor(
                        out=mtmp[:, rs * 128 : (rs + 1) * 128],
                        in0=acol_t[:, ci, rs * 128 : (rs + 1) * 128],
                        in1=pst[:],
                        op=mybir.AluOpType.add,
                    )
                # relu into m_all; accum_out = partial column sums for free
                nc.scalar.activation(
                    out=m_all[:, ct, :],
                    in_=mtmp[:],
                    func=mybir.ActivationFunctionType.Relu,
                    accum_out=pd[:, ct : ct + 1],
                )

        # ---- AllReduce the partial degrees -> degrees of every node ----
        nc.sync.dma_start(out=pd_in[:], in_=pd[:])
        nc.gpsimd.collective_compute(
            "AllReduce",
            mybir.AluOpType.add,
            replica_groups=[list(range(N_CORES))],
            ins=[pd_in.opt()],
            outs=[pd_out.opt()],
        )
        # ---- local row sums over the (unscaled!) M tiles, before scaling ----
        d_ps = ps_d.tile([1, RB], f32)
        for ct in range(NT):
            nc.tensor.matmul(
                d_ps[:],
                lhsT=ones_bf[:],
                rhs=m_all[:, ct, :],
                start=(ct == 0),
                stop=(ct == NT - 1),
            )
        # ---- dis[c] for all nodes; fold into M in place ----
        d_all = singles.tile([128, NT], f32, name="d_all", tag="d_all")
        nc.sync.dma_start(out=d_all[:], in_=pd_out[:])
        _rsqrt_guarded(nc, mybir, singles, d_all, ones32, disall, NT, "all")
        for ct in range(NT):
            nc.vector.tensor_scalar_mul(
                m_all[:, ct, :], m_all[:, ct, :], disall[:, ct : ct + 1]
            )

        # ---- local dis[R] for the epilogue ----
        d_sb = singles.tile([1, RB], f32)
        nc.scalar.copy(d_sb[:], d_ps[:])
        dps_t = ps_tr.tile([128, 4], f32, tag="pst")
        for rs in range(RS):
            nc.tensor.transpose(
                dps_t[:, rs : rs + 1],
                d_sb[0:1, rs * 128 : (rs + 1) * 128],
                identity[0:1, 0:1],
            )
        dT = singles.tile([128, 4], f32)
        nc.vector.tensor_copy(out=dT[:], in_=dps_t[:])
        _rsqrt_guarded(nc, mybir, singles, dT, ones4, dis4, RS, "loc")
        nc.vector.tensor_scalar_mul(neg_dis4[:], dis4[:], -1.0)

    # ---------- Phase C: out[R] = relu(HW[R] - dis_r*(M'^T @ HWg)) ---------
    with (
        tc.tile_pool(name="gp", bufs=2) as gp,
        tc.tile_pool(name="epi", bufs=4) as epi,
        tc.tile_pool(name="outp", bufs=4) as outp,
        tc.tile_pool(name="ps_mm", bufs=4, space="PSUM") as ps_mm,
    ):
        for pair in range(B // 2):
            g_pair = gp.tile([128, NT, 2, F], bf16, tag="g_pair")
            for rank in range(N_CORES):
                for bp in range(2):
                    b = pair * 2 + bp
                    nc.sync.dma_start(
                        out=g_pair[:, rank * 4 : (rank + 1) * 4, bp, :],
                        in_=ag_out[rank, b].rearrange("(j p) f -> p j f", p=128),
                    )
            for rs in range(RS):
                pmm = ps_mm.tile([128, 2, F], f32, tag="pmm")
                for ct in range(NT):
                    nc.tensor.matmul(
                        pmm[:],
                        lhsT=m_all[:, ct, rs * 128 : (rs + 1) * 128],
                        rhs=g_pair[:, ct, :, :],
                        start=(ct == 0),
                        stop=(ct == NT - 1),
                    )
                for bp in range(2):
                    b = pair * 2 + bp
                    t1 = epi.tile([128, F], f32, tag="t1")
                    nc.vector.scalar_tensor_tensor(
                        out=t1[:],
                        in0=pmm[:, bp, :],
                        scalar=neg_dis4[:, rs : rs + 1],
                        in1=hwr[:, b, rs, :],
                        op0=mybir.AluOpType.mult,
                        op1=mybir.AluOpType.add,
                    )
                    o_t = outp.tile([128, F], f32, tag="o_t")
                    nc.scalar.activation(
                        o_t[:], t1[:], mybir.ActivationFunctionType.Relu
                    )
                    nc.sync.dma_start(
                        out=out_r[b, rs * 128 : (rs + 1) * 128, :], in_=o_t[:]
                    )


def _rsqrt_guarded(nc, mybir, singles, d_t, ones_t, out_t, width, suffix):
    """out = where(d > 0, 1/sqrt(d), 0) elementwise on a [128, width] tile."""
    f32 = mybir.dt.float32
    mask_u = singles.tile([128, width], mybir.dt.uint8, name=f"mask_u_{suffix}")
    nc.vector.tensor_scalar(
        out=mask_u[:], in0=d_t[:], scalar1=0.0, scalar2=None,
        op0=mybir.AluOpType.is_gt,
    )
    mask_f = singles.tile([128, width], f32, name=f"mask_f_{suffix}")
    nc.vector.tensor_scalar(
        out=mask_f[:], in0=d_t[:], scalar1=0.0, scalar2=None,
        op0=mybir.AluOpType.is_gt,
    )
    dsafe = singles.tile([128, width], f32, name=f"dsafe_{suffix}")
    nc.vector.select(dsafe[:], mask_u[:], d_t[:], ones_t[:])
    rcp = singles.tile([128, width], f32, name=f"rcp_{suffix}")
    nc.vector.reciprocal(rcp[:], dsafe[:])
    srt = singles.tile([128, width], f32, name=f"srt_{suffix}")
    nc.scalar.activation(srt[:], rcp[:], mybir.ActivationFunctionType.Sqrt)
    nc.vector.tensor_tensor(
        out=out_t[:], in0=srt[:], in1=mask_f[:], op=mybir.AluOpType.mult
    )


_NC_CACHE = None


def kernel(H, W, A):
    global _NC_CACHE
    from concourse.bass_utils import run_bass_kernel_spmd

    H = np.asarray(H, dtype=np.float32)
    W = np.asarray(W, dtype=np.float32)
    A = np.asarray(A, dtype=np.float32)

    if _NC_CACHE is None:
        _NC_CACHE = _build_kernel()
    nc = _NC_CACHE

    in_maps = []
    for c in range(N_CORES):
        r0, r1 = c * RB, (c + 1) * RB
        in_maps.append(
            {
                "Arows": np.ascontiguousarray(A[r0:r1, :]),
                "Acols": np.ascontiguousarray(A[:, r0:r1]),
                "Hr": np.ascontiguousarray(H[:, r0:r1, :]),
                "W": W,
            }
        )

    res = run_bass_kernel_spmd(nc, in_maps, list(range(N_CORES)))

    out = np.empty((B, N, F), dtype=np.float32)
    for c in range(N_CORES):
        out[:, c * RB : (c + 1) * RB, :] = res.results[c]["OutR"]
    return out


if __name__ == "__main__":
    rng = np.random.default_rng(0)
    H = rng.standard_normal((B, N, F)).astype(np.float32)
    W = rng.standard_normal((F, F)).astype(np.float32) / 16.0
    A = rng.standard_normal((N, N)).astype(np.float32) * 0.0262
    out = kernel(H, W, A)
    print("kernel ran, out shape", out.shape)


# revision 13
# speedup vs baseline: 1.5794x; 1.4195x over previous
"""DGC layer (graph conv with normalized Laplacian) on 8 Trainium2 NeuronCores.

Computes out = relu((I - D^-1/2 A_norm D^-1/2) @ (H @ W)) with
A_norm = relu((A + A.T)/2), sharded row-wise over 8 cores.

Math notes (per core, row block R of 512 rows):
  - A_norm is symmetric, so its column block A_norm[:, R] equals
    A_norm[R, :].T - exactly the layout the tensor engine wants for
    contracting over the full node dim.
  - The 0.5 symmetrization factor cancels out of D^-1/2 A_norm D^-1/2,
    so we use S = relu(A + A.T) and d = rowsum(S) instead.
  - out[R] = relu(HW[R] - dis[R] * (M'.T @ HWg)) where M' = dis_c * S[:, R]
    (bf16, column scaling folded into the stationary operand), HWg = the
    AllGathered unscaled bf16 HW, and HW[R] kept in fp32 so the dominant
    term has full precision.
  - The HW AllGather only depends on H @ W, so it runs concurrently with
    the whole A_norm build. Degrees for all nodes come from AllReduces of
    per-core partial column sums (free via the relu's accum_out), split in
    two halves so the first half of the big matmul can start while the
    second half of M is still being built.
"""

import sys

sys.path.insert(0, "/opt/trn_rl_repo")

import numpy as np

B, N, F = 8, 4096, 256
N_CORES = 8
RB = N // N_CORES          # 512 rows per core
NT = N // 128              # 32 contraction tiles of 128
NH = NT // 2               # 16 tiles per degree-reduce half
RS = RB // 128             # 4 row subtiles per core
FT = F // 128              # 2 f_in tiles
_SIM_LOCAL_AG = False      # analyze.py sets True (fake collective, Local DRAM)


def _build_kernel(repeat=1):
    import concourse.mybir as mybir
    import concourse.tile as tile
    from concourse import bacc
    from concourse.masks import make_identity

    f32 = mybir.dt.float32
    bf16 = mybir.dt.bfloat16

    nc = bacc.Bacc(num_devices=N_CORES)

    arows = nc.declare_dram_parameter("Arows", [RB, N], f32, isOutput=False)
    acols = nc.declare_dram_parameter("Acols", [N, RB], f32, isOutput=False)
    hr = nc.declare_dram_parameter("Hr", [B, RB, F], f32, isOutput=False)
    w = nc.declare_dram_parameter("W", [F, F], f32, isOutput=False)
    out_r = nc.declare_dram_parameter("OutR", [B, RB, F], f32, isOutput=True)

    with tile.TileContext(nc) as tc:
        with (
            tc.tile_pool(name="singles", bufs=1) as singles,
            tc.tile_pool(name="persist", bufs=1) as persist,
            tc.tile_pool(name="dram", bufs=1, space="DRAM") as dram,
        ):
            identity = singles.tile([128, 128], f32)
            make_identity(nc, identity[:])
            ones_bf = singles.tile([128, 1], bf16)
            nc.vector.memset(ones_bf[:], 1.0)
            ones4 = singles.tile([128, 4], f32)
            nc.vector.memset(ones4[:], 1.0)
            ones32 = singles.tile([128, NT], f32)
            nc.vector.memset(ones32[:], 1.0)

            # W in [f_in (part), ft, f_out] layout, float32r for the fast matmul
            w_sb = singles.tile([128, FT, F], mybir.dt.float32r)
            nc.sync.dma_start(
                out=w_sb[:],
                in_=w.rearrange("(t p) o -> p t o", p=128).bitcast(
                    mybir.dt.float32r
                ),
            )

            # persistent blocks
            m_all = persist.tile([128, NT, RB], bf16)       # S[:, R], c on partitions
            m8 = persist.tile([128, NT, RB], mybir.dt.float8e4)  # 64*dis_c*S
            hwr = persist.tile([128, B, RS, F], f32)        # HW[b, R, f] fp32
            pd = persist.tile([128, NT], f32)               # partial col sums
            dis4 = persist.tile([128, 4], f32)              # dis[R] as [p, rs]
            neg_dis4 = persist.tile([128, 4], f32)
            disall = persist.tile([128, NT], f32)           # dis[c] as [p, ct]

            for _rep in range(repeat):
                _kernel_body(nc, tc, mybir, singles, dram,
                             identity, ones_bf, ones4, ones32, w_sb,
                             m_all, m8, hwr, pd, dis4, neg_dis4, disall,
                             arows, acols, hr, out_r)

    nc.compile()
    return nc


def _kernel_body(nc, tc, mybir, singles, dram,
                 identity, ones_bf, ones4, ones32, w_sb,
                 m_all, m8, hwr, pd, dis4, neg_dis4, disall,
                 arows, acols, hr, out_r):
    f32 = mybir.dt.float32
    f32r = mybir.dt.float32r
    bf16 = mybir.dt.bfloat16

    fp8 = mybir.dt.float8e4
    ag_in = dram.tile([B, RB, F], fp8, tag="ag_in")
    ag_out = dram.tile(
        [N_CORES, B, RB, F],
        fp8,
        addr_space="Local" if _SIM_LOCAL_AG else "Shared",
        tag="ag_out",
    )
    pd_in = dram.tile([128, NT], f32, tag="pd_in")
    pd_out = dram.tile(
        [128, NT], f32,
        addr_space="Local" if _SIM_LOCAL_AG else "Shared", tag="pd_out",
    )

    # ---------- Phase A: HW = H @ W (float32r), cast to bf16, AllGather ----
    with (
        tc.tile_pool(name="hp", bufs=2) as hp,
        tc.tile_pool(name="htp", bufs=2) as htp,
        tc.tile_pool(name="ps_tr_h", bufs=2, space="PSUM") as ps_tr_h,
        tc.tile_pool(name="ps_hw", bufs=2, space="PSUM") as ps_hw,
    ):
        for b in range(B):
            h_b = hp.tile([128, RS, F], f32, tag="h_b")
            nc.sync.dma_start(
                out=h_b[:], in_=hr[b].rearrange("(j p) f -> p j f", p=128)
            )
            ht_b = htp.tile([128, FT, RB], f32r, tag="ht_b")
            for rs in range(RS):
                for ft in range(FT):
                    pst = ps_tr_h.tile([128, 128], f32, tag="psth")
                    nc.tensor.transpose(
                        pst[:],
                        h_b[:, rs, ft * 128 : (ft + 1) * 128],
                        identity[:],
                    )
                    nc.scalar.copy(
                        ht_b[:, ft, rs * 128 : (rs + 1) * 128], pst[:]
                    )
            for rs in range(RS):
                phw = ps_hw.tile([128, F], f32, tag="phw")
                for ft in range(FT):
                    nc.tensor.matmul(
                        phw[:],
                        lhsT=ht_b[:, ft, rs * 128 : (rs + 1) * 128],
                        rhs=w_sb[:, ft, :],
                        start=(ft == 0),
                        stop=(ft == FT - 1),
                    )
                nc.vector.tensor_copy(out=hwr[:, b, rs, :], in_=phw[:])
            # bf16 cast during DMA (SWDGE) straight into the gather input
            nc.gpsimd.dma_start(
                out=ag_in[b].rearrange("(j p) f -> p j f", p=128),
                in_=hwr[:, b, :, :],
            )

    nc.gpsimd.collective_compute(
        "AllGather",
        mybir.AluOpType.bypass,
        replica_groups=[list(range(N_CORES))],
        ins=[ag_in.opt()],
        outs=[ag_out.opt()],
    )

    # ---------- Phase B: M = relu(Acols + Arows^T) bf16, degrees ----------
    with (
        tc.tile_pool(name="arst", bufs=1) as arst,
        tc.tile_pool(name="acolp", bufs=3) as acolp,
        tc.tile_pool(name="mtmpp", bufs=3) as mtmpp,
        tc.tile_pool(name="ps_tr", bufs=4, space="PSUM") as ps_tr,
        tc.tile_pool(name="ps_d", bufs=1, space="PSUM") as ps_d,
    ):
        ars_sb = arst.tile([128, RS, N], f32)
        for rs in range(RS):
            nc.sync.dma_start(
                out=ars_sb[:, rs, :], in_=arows[rs * 128 : (rs + 1) * 128, :]
            )

        CC = 4                      # acol tiles per DMA chunk (1 MiB)
        for cc in range(NT // CC):
            acol_t = acolp.tile([128, CC, RB], f32, tag="acol")
            nc.sync.dma_start(
                out=acol_t[:],
                in_=acols[cc * CC * 128 : (cc + 1) * CC * 128, :]
                .rearrange("(c p) r -> p c r", p=128),
            )
            for ci in range(CC):
                ct = cc * CC + ci
                mtmp = mtmpp.tile([128, RB], bf16, tag="mtmp")
                for rs in range(RS):
                    pst = ps_tr.tile([128, 128], f32, tag="pst")
                    nc.tensor.transpose(
                        pst[:],
                        ars_sb[:, rs, ct * 128 : (ct + 1) * 128],
                        identity[:],
                    )
                    nc.vector.tensor_tensor(
                        out=mtmp[:, rs * 128 : (rs + 1) * 128],
                        in0=acol_t[:, ci, rs * 128 : (rs + 1) * 128],
                        in1=pst[:],
                        op=mybir.AluOpType.add,
                    )
                # relu into m_all; accum_out = partial column sums for free
                nc.scalar.activation(
                    out=m_all[:, ct, :],
                    in_=mtmp[:],
                    func=mybir.ActivationFunctionType.Relu,
                    accum_out=pd[:, ct : ct + 1],
                )

        # ---- AllReduce the partial degrees -> degrees of every node ----
        nc.sync.dma_start(out=pd_in[:], in_=pd[:])
        nc.gpsimd.collective_compute(
            "AllReduce",
            mybir.AluOpType.add,
            replica_groups=[list(range(N_CORES))],
            ins=[pd_in.opt()],
            outs=[pd_out.opt()],
        )
        # ---- local row sums over the (unscaled!) M tiles, before scaling ----
        d_ps = ps_d.tile([1, RB], f32)
        for ct in range(NT):
            nc.tensor.matmul(
                d_ps[:],
                lhsT=ones_bf[:],
                rhs=m_all[:, ct, :],
                start=(ct == 0),
                stop=(ct == NT - 1),
            )
        # ---- dis[c] for all nodes; fold into M in place ----
        d_all = singles.tile([128, NT], f32, name="d_all", tag="d_all")
        nc.sync.dma_start(out=d_all[:], in_=pd_out[:])
        _rsqrt_guarded(nc, mybir, singles, d_all, ones32, disall, NT, "all",
                       scale=64.0)
        for ct in range(NT):
            nc.vector.tensor_scalar_mul(
                m8[:, ct, :], m_all[:, ct, :], disall[:, ct : ct + 1]
            )

        # ---- local dis[R] for the epilogue ----
        d_sb = singles.tile([1, RB], f32)
        nc.scalar.copy(d_sb[:], d_ps[:])
        dps_t = ps_tr.tile([128, 4], f32, tag="pst")
        for rs in range(RS):
            nc.tensor.transpose(
                dps_t[:, rs : rs + 1],
                d_sb[0:1, rs * 128 : (rs + 1) * 128],
                identity[0:1, 0:1],
            )
        dT = singles.tile([128, 4], f32)
        nc.vector.tensor_copy(out=dT[:], in_=dps_t[:])
        _rsqrt_guarded(nc, mybir, singles, dT, ones4, dis4, RS, "loc")
        nc.vector.tensor_scalar_mul(neg_dis4[:], dis4[:], -1.0 / 64.0)

    # ---------- Phase C: out[R] = relu(HW[R] - dis_r*(M'^T @ HWg)) ---------
    with (
        tc.tile_pool(name="gp", bufs=2) as gp,
        tc.tile_pool(name="epi", bufs=4) as epi,
        tc.tile_pool(name="outp", bufs=4) as outp,
        tc.tile_pool(name="ps_mm", bufs=4, space="PSUM") as ps_mm,
    ):
        for pair in range(B // 2):
            g_pair = gp.tile([128, NT, 2, F], fp8, tag="g_pair")
            for rank in range(N_CORES):
                for bp in range(2):
                    b = pair * 2 + bp
                    nc.sync.dma_start(
                        out=g_pair[:, rank * 4 : (rank + 1) * 4, bp, :],
                        in_=ag_out[rank, b].rearrange("(j p) f -> p j f", p=128),
                    )
            for rs in range(RS):
                pmm = ps_mm.tile([128, 2, F], f32, tag="pmm")
                for t in range(NT // 2):
                    nc.tensor.matmul(
                        pmm[:],
                        lhsT=m8[:, 2 * t : 2 * t + 2, rs * 128 : (rs + 1) * 128],
                        rhs=g_pair[:, 2 * t : 2 * t + 2, :, :],
                        start=(t == 0),
                        stop=(t == NT // 2 - 1),
                        perf_mode=mybir.MatmulPerfMode.DoubleRow,
                    )
                for bp in range(2):
                    b = pair * 2 + bp
                    t1 = epi.tile([128, F], f32, tag="t1")
                    nc.vector.scalar_tensor_tensor(
                        out=t1[:],
                        in0=pmm[:, bp, :],
                        scalar=neg_dis4[:, rs : rs + 1],
                        in1=hwr[:, b, rs, :],
                        op0=mybir.AluOpType.mult,
                        op1=mybir.AluOpType.add,
                    )
                    o_t = outp.tile([128, F], f32, tag="o_t")
                    nc.scalar.activation(
                        o_t[:], t1[:], mybir.ActivationFunctionType.Relu
                    )
                    nc.sync.dma_start(
                        out=out_r[b, rs * 128 : (rs + 1) * 128, :], in_=o_t[:]
                    )


def _rsqrt_guarded(nc, mybir, singles, d_t, ones_t, out_t, width, suffix,
                   scale=1.0):
    """out = scale * where(d > 0, 1/sqrt(d), 0) on a [128, width] tile."""
    f32 = mybir.dt.float32
    mask_u = singles.tile([128, width], mybir.dt.uint8, name=f"mask_u_{suffix}")
    nc.vector.tensor_scalar(
        out=mask_u[:], in0=d_t[:], scalar1=0.0, scalar2=None,
        op0=mybir.AluOpType.is_gt,
    )
    mask_f = singles.tile([128, width], f32, name=f"mask_f_{suffix}")
    nc.vector.tensor_scalar(
        out=mask_f[:], in0=d_t[:], scalar1=0.0, scalar2=None,
        op0=mybir.AluOpType.is_gt,
    )
    dsafe = singles.tile([128, width], f32, name=f"dsafe_{suffix}")
    nc.vector.select(dsafe[:], mask_u[:], d_t[:], ones_t[:])
    rcp = singles.tile([128, width], f32, name=f"rcp_{suffix}")
    nc.vector.reciprocal(rcp[:], dsafe[:])
    srt = singles.tile([128, width], f32, name=f"srt_{suffix}")
    nc.scalar.activation(srt[:], rcp[:], mybir.ActivationFunctionType.Sqrt)
    nc.vector.scalar_tensor_tensor(
        out=out_t[:], in0=srt[:], scalar=scale, in1=mask_f[:],
        op0=mybir.AluOpType.mult, op1=mybir.AluOpType.mult,
    )


_NC_CACHE = None


def kernel(H, W, A):
    global _NC_CACHE
    from concourse.bass_utils import run_bass_kernel_spmd

    H = np.asarray(H, dtype=np.float32)
    W = np.asarray(W, dtype=np.float32)
    A = np.asarray(A, dtype=np.float32)

    if _NC_CACHE is None:
        _NC_CACHE = _build_kernel()
    nc = _NC_CACHE

    in_maps = []
    for c in range(N_CORES):
        r0, r1 = c * RB, (c + 1) * RB
        in_maps.append(
            {
                "Arows": np.ascontiguousarray(A[r0:r1, :]),
                "Acols": np.ascontiguousarray(A[:, r0:r1]),
                "Hr": np.ascontiguousarray(H[:, r0:r1, :]),
                "W": W,
            }
        )

    res = run_bass_kernel_spmd(nc, in_maps, list(range(N_CORES)))

    out = np.empty((B, N, F), dtype=np.float32)
    for c in range(N_CORES):
        out[:, c * RB : (c + 1) * RB, :] = res.results[c]["OutR"]
    return out


if __name__ == "__main__":
    rng = np.random.default_rng(0)
    H = rng.standard_normal((B, N, F)).astype(np.float32)
    W = rng.standard_normal((F, F)).astype(np.float32) / 16.0
    A = rng.standard_normal((N, N)).astype(np.float32) * 0.0262
    out = kernel(H, W, A)
    print("kernel ran, out shape", out.shape)
